# revision 40
# baseline (speedup 1.0000x reference)
"""Trainium2 Bass kernel for DGMoLE (dense-gated mixture of LoRA experts).

Computes, for x:[B,S,Din], W_base:[Dout,Din], b_base:[Dout], W_router:[E,Din],
b_router:[E], lora_A:[E,Din,R], lora_B:[E,R,Dout]:

    base   = x @ W_base.T + b_base
    wts    = sparsemax(x @ W_router.T + b_router)
    h      = einsum('td,edr->ter', x, lora_A)
    out    = base + einsum('ter,te,ero->to', h, wts, lora_B)

Sharding over 8 NeuronCores: 4 token-quarters x 2 Dout-halves.

v2: all operand layout work is done host-side in numpy (untimed):
x is pre-transposed into per-tile [d,t] bf16 blocks, W into [d,o] bf16,
router+lora_A packed into one [d, 8+128] rhs, biases pre-broadcast.
The device program is pure matmul streaming: per 128-token tile, one
c-loop of 32x(4 matmuls of 512 cols) accumulating base output, with the
next tile's router/h matmuls and this tile's hw transpose interleaved.
No PE transposes of x/W, no dtype-converting DMA, no startup W build.
"""

import sys

sys.path.insert(0, "/opt/trn_rl_repo")

import numpy as np
import ml_dtypes

from concourse import bacc, tile, mybir
from concourse.bass_utils import run_bass_kernel_spmd

f32 = mybir.dt.float32
bf16 = mybir.dt.bfloat16
Add = mybir.AluOpType.add
Mult = mybir.AluOpType.mult
Max = mybir.AluOpType.max
Min = mybir.AluOpType.min
IsGt = mybir.AluOpType.is_gt
Sub = mybir.AluOpType.subtract

# Problem dims (hardcoded per spec).
B, S, D, O = 8, 2048, 4096, 4096
E, R = 8, 16
ER = E * R  # 128
N_CORES = 8
TQ = 4          # token quarters
OH = 2          # output halves
T_CORE = B * S // TQ      # 4096 tokens per core
O_CORE = O // OH          # 2048 output dims per core
NT = T_CORE // 128        # 32 token tiles
NC_D = D // 128           # 32 contraction chunks
RH = E + ER               # router+h fused rhs width = 136

_CACHE = {}


def _build(trace_sim=False):
    if "nc" in _CACHE:
        return _CACHE["nc"]

    nc = bacc.Bacc("TRN2", target_bir_lowering=False, debug=False,
                   num_devices=N_CORES)
    # Host-packed inputs (see make_in_maps).
    xt_d = nc.dram_tensor("xt", [NT, 128, NC_D * 128], bf16,
                          kind="ExternalInput").ap()   # [tile, d_in_chunk, (c,t)]
    wt_d = nc.dram_tensor("wt", [128, NC_D, O_CORE], bf16,
                          kind="ExternalInput").ap()   # [d_in_chunk, c, o]
    comb_d = nc.dram_tensor("comb", [128, NC_D, RH], bf16,
                            kind="ExternalInput").ap()  # [d, c, (router|loraA)]
    bcat_d = nc.dram_tensor("bcat", [ER, O_CORE], bf16,
                            kind="ExternalInput").ap()
    bb_d = nc.dram_tensor("bb", [128, O_CORE], f32, kind="ExternalInput").ap()
    brb_d = nc.dram_tensor("brb", [128, E], f32, kind="ExternalInput").ap()
    kb_d = nc.dram_tensor("kb", [128, E], f32, kind="ExternalInput").ap()
    id_d = nc.dram_tensor("ident", [128, 128], bf16, kind="ExternalInput").ap()
    out_d = nc.dram_tensor("out", [T_CORE, O_CORE], f32,
                           kind="ExternalOutput").ap()

    with tile.TileContext(nc, trace_sim=trace_sim) as tc:
        with (
            tc.tile_pool(name="const", bufs=1) as cpool,
            tc.tile_pool(name="xt", bufs=5) as xtpool,
            tc.tile_pool(name="small", bufs=2) as smpool,
            tc.tile_pool(name="outs", bufs=3) as outpool,
            tc.tile_pool(name="ps", bufs=1, space="PSUM") as pspool,
        ):
            # ---------------- constants (all straight DMA) ----------------
            # Issue order matters at startup: rh(0) needs xt0+comb, og(0)
            # needs wt chunks in order; everything shares ~350 GB/s.
            def load_xt(i, split=1):
                t = xtpool.tile([128, NC_D * 128], bf16, tag="xt", name="xtt")
                step = NC_D * 128 // split
                for s in range(split):
                    nc.sync.dma_start(t[:, s * step:(s + 1) * step],
                                      xt_d[i, :, s * step:(s + 1) * step])
                return t

            ident = cpool.tile([128, 128], bf16)
            nc.sync.dma_start(ident[:], id_d[:])
            xts = {}
            comb = cpool.tile([128, NC_D * RH], bf16)
            xt0 = xtpool.tile([128, NC_D * 128], bf16, tag="xt", name="xtt")
            for g in range(2):
                nc.sync.dma_start(xt0[:, g * 2048:(g + 1) * 2048],
                                  xt_d[0, :, g * 2048:(g + 1) * 2048])
                nc.sync.dma_start(
                    comb[:, g * 16 * RH:(g + 1) * 16 * RH].rearrange(
                        "p (c f) -> p c f", f=RH),
                    comb_d[:, g * 16:(g + 1) * 16, :])
            xts[0] = xt0
            xts[1] = load_xt(1)
            xts[2] = load_xt(2)

            # W^T chunks: one tile per contraction chunk so matmuls only
            # wait on the chunk they read.  x tiles 1-3 are interspersed so
            # their router/h chains can fill og(0)'s W-arrival stalls.
            wt = []
            for c in range(NC_D):
                wt_c = cpool.tile([128, O_CORE], bf16, tag=f"wt{c}",
                                  name=f"wt{c}")
                nc.gpsimd.dma_start(wt_c[:], wt_d[:, c, :])
                wt.append(wt_c)
            bcat = cpool.tile([128, O_CORE], bf16)
            nc.sync.dma_start(bcat[:], bcat_d[:])
            bb = cpool.tile([128, O_CORE], f32)
            nc.sync.dma_start(bb[:], bb_d[:])
            xts[3] = load_xt(3)
            brb = cpool.tile([128, E], f32)
            nc.sync.dma_start(brb[:], brb_d[:])
            kb = cpool.tile([128, E], f32)
            nc.sync.dma_start(kb[:], kb_d[:])
            rh_ps = {}
            hw_tiles = {}

            def emit_sparsemax(i, rh):
                """DVE chain: rh (SBUF copy) logits -> weights -> hw."""
                z = smpool.tile([128, E], f32, tag="z", name="z")
                nc.vector.tensor_tensor(z[:], rh[:, 0:E], brb[:], op=Add)
                zs = smpool.tile([128, E], f32, tag="zs", name="zs")
                nc.vector.tensor_copy(zs[:], z[:])
                tmp = smpool.tile([128, E], f32, tag="tmp", name="tmp")
                # Batcher sort-8 (descending), comparators batched per stage
                # via strided access patterns: 6 stages x 3 ops.
                z2 = zs[:].rearrange("p (f two) -> p f two", two=2)
                z4 = zs[:].rearrange("p (g two f) -> p g two f", two=2, f=2)
                z8 = zs[:].rearrange("p (g f) -> p g f", f=4)
                z6 = zs[:, 1:7].rearrange("p (f two) -> p f two", two=2)
                t41 = tmp[:, 0:4].rearrange("p (f one) -> p f one", one=1)
                t42 = tmp[:, 0:4].rearrange("p (g one f) -> p g one f",
                                            one=1, f=2)
                t21 = tmp[:, 0:2].rearrange("p (f one) -> p f one", one=1)
                t31 = tmp[:, 0:3].rearrange("p (f one) -> p f one", one=1)
                stages = [
                    (z2[:, :, 0:1], z2[:, :, 1:2], t41),        # (01)(23)(45)(67)
                    (z4[:, :, 0:1, :], z4[:, :, 1:2, :], t42),  # (02)(13)(46)(57)
                    (z8[:, :, 1:2], z8[:, :, 2:3], t21),        # (12)(56)
                    (zs[:, 0:4], zs[:, 4:8], tmp[:, 0:4]),      # (04)(15)(26)(37)
                    (zs[:, 2:4], zs[:, 4:6], tmp[:, 0:2]),      # (24)(35)
                    (z6[:, :, 0:1], z6[:, :, 1:2], t31),        # (12)(34)(56)
                ]
                for (a_, b_, t_) in stages:
                    nc.vector.tensor_tensor(t_, a_, b_, op=Min)
                    nc.vector.tensor_tensor(a_, a_, b_, op=Max)
                    nc.vector.tensor_copy(b_, t_)
                # inclusive prefix sum along E (doubling, 6 ops)
                c1 = smpool.tile([128, E], f32, tag="c1", name="c1")
                c2 = smpool.tile([128, E], f32, tag="c2", name="c2")
                cum = smpool.tile([128, E], f32, tag="cum", name="cum")
                nc.vector.tensor_tensor(c1[:, 1:8], zs[:, 1:8], zs[:, 0:7],
                                        op=Add)
                nc.vector.tensor_copy(c1[:, 0:1], zs[:, 0:1])
                nc.vector.tensor_tensor(c2[:, 2:8], c1[:, 2:8], c1[:, 0:6],
                                        op=Add)
                nc.vector.tensor_copy(c2[:, 0:2], c1[:, 0:2])
                nc.vector.tensor_tensor(cum[:, 4:8], c2[:, 4:8], c2[:, 0:4],
                                        op=Add)
                nc.vector.tensor_copy(cum[:, 0:4], c2[:, 0:4])
                # support: 1 + k*z_(k) > cum_k
                kz1 = smpool.tile([128, E], f32, tag="kz1", name="kz1")
                nc.vector.tensor_tensor(kz1[:], zs[:], kb[:], op=Mult)
                nc.vector.tensor_scalar_add(kz1[:], kz1[:], 1.0)
                supp = smpool.tile([128, E], f32, tag="supp", name="supp")
                nc.vector.tensor_tensor(supp[:], kz1[:], cum[:], op=IsGt)
                kz = smpool.tile([128, 1], f32, tag="kz", name="kz")
                nc.vector.tensor_reduce(kz[:], supp[:],
                                        axis=mybir.AxisListType.X, op=Add)
                zsup = smpool.tile([128, E], f32, tag="zsup", name="zsup")
                tsum = smpool.tile([128, 1], f32, tag="tsum", name="tsum")
                nc.vector.tensor_tensor(zsup[:], zs[:], supp[:], op=Mult)
                nc.vector.tensor_reduce(tsum[:], zsup[:],
                                        axis=mybir.AxisListType.X, op=Add)
                nc.vector.tensor_scalar_add(tsum[:], tsum[:], -1.0)
                rk = smpool.tile([128, 1], f32, tag="rk", name="rk")
                nc.vector.reciprocal(rk[:], kz[:])
                tau = smpool.tile([128, 1], f32, tag="tau", name="tau")
                nc.vector.tensor_tensor(tau[:], tsum[:], rk[:], op=Mult)
                wts = smpool.tile([128, E], f32, tag="wts", name="wts")
                nc.vector.tensor_scalar(wts[:], z[:], tau[:], 0.0,
                                        op0=Sub, op1=Max)
                # hw = h * wts (bf16): one op, wts broadcast across ranks
                hw = smpool.tile([128, ER], bf16, tag="hw", name="hw",
                                 bufs=4)
                nc.vector.tensor_tensor(
                    hw[:].rearrange("p (e r) -> p e r", r=R),
                    rh[:, E:RH].rearrange("p (e r) -> p e r", r=R),
                    wts[:].rearrange("p (e one) -> p e one",
                                     one=1).broadcast_to([128, E, R]),
                    op=Mult)
                hw_tiles[i] = hw

            # ---------------- rh chain machinery ----------------
            # Router/h chains run as a cursor-driven pipeline up to 2 tiles
            # ahead of the og loop.  Each completed chain is copied to SBUF
            # (freeing the single rh psum bank) and its sparsemax emitted.
            cursor = [0, 0]

            def emit_rh_mm():
                j, k = cursor
                if k == 0:
                    rh_ps[j] = pspool.tile([128, RH], f32, tag="rh", bufs=1,
                                           name="rhn")
                nc.tensor.matmul(rh_ps[j][:], xts[j][:, k * 128:(k + 1) * 128],
                                 comb[:, k * RH:(k + 1) * RH],
                                 start=(k == 0), stop=(k == NC_D - 1))
                if k == NC_D - 1:
                    rhc = smpool.tile([128, RH], f32, tag="rhc", name="rhc")
                    nc.vector.tensor_copy(rhc[:], rh_ps[j][:])
                    emit_sparsemax(j, rhc)
                    cursor[0], cursor[1] = j + 1, 0
                else:
                    cursor[1] = k + 1

            # ---------------- prologue ----------------
            # PE warm-up: dummy transposes on a memset tile (no DMA dep)
            # keep the HAM activity window busy from ~0.5us until the first
            # input DMAs land, so rh(0)/og(0) run at 2.4 GHz, not 1.2.
            junk = cpool.tile([128, 128], bf16, name="junk")
            nc.vector.memset(junk[:], 0.0)
            warm = pspool.tile([128, 128], f32, tag="tr", bufs=1,
                               name="warm")
            for _ in range(120):
                nc.tensor.matmul(warm[:], junk[:], junk[:],
                                 start=True, stop=True)

            # ---------------- main token loop ----------------
            for i in range(NT):
                if i + 4 < NT:
                    xts[i + 4] = load_xt(i + 4)
                xt_i = xts[i]
                hwT = None
                rh_cap = min(i + 3 if i == 0 else i + 2, NT - 1)
                hw_c = 22 if i == 0 else 14
                # The last tile runs as two sequential o-half passes so the
                # first half's evacuation+DMA overlaps the second's matmuls,
                # trimming the program tail.
                passes = [(0,), (1,)] if i == NT - 1 else [(0, 1)]
                for halves in passes:
                    accs = {
                        h: pspool.tile([128, 1024], f32, tag="og", bufs=3,
                                       name=f"acc{h}")
                        for h in halves
                    }
                    for c in range(NC_D):
                        lhs = xt_i[:, c * 128:(c + 1) * 128]
                        w_c = wt[c]
                        st = (c == 0)
                        for h in halves:
                            o0 = h * 1024
                            nc.tensor.matmul(
                                accs[h][:, 0:512], lhs,
                                w_c[:, o0:o0 + 512], start=st, stop=False)
                            nc.tensor.matmul(
                                accs[h][:, 512:1024], lhs,
                                w_c[:, o0 + 512:o0 + 1024],
                                start=st, stop=False)
                        if c == hw_c and hwT is None:
                            # transpose hw(i) for the lora_B matmuls below
                            trp = pspool.tile([128, 128], bf16, tag="tr",
                                              bufs=1, name="trp")
                            nc.tensor.transpose(trp[:], hw_tiles[i][:],
                                                ident[:])
                            hwT = smpool.tile([128, ER], bf16, tag="hwT",
                                              name="hwT")
                            nc.vector.tensor_copy(hwT[:], trp[:])
                        if c >= (1 if i == 0 else 4):
                            behind = cursor[0] <= i + 1
                            quota = (5 if i == 0 else 3) if behind else 2
                            n = 0
                            while cursor[0] <= rh_cap and n < quota:
                                emit_rh_mm()
                                n += 1
                    # close base accumulation with the lora_B contribution
                    for h in halves:
                        o0 = h * 1024
                        nc.tensor.matmul(accs[h][:, 0:512], hwT[:],
                                         bcat[:, o0:o0 + 512],
                                         start=False, stop=True)
                        nc.tensor.matmul(accs[h][:, 512:1024], hwT[:],
                                         bcat[:, o0 + 512:o0 + 1024],
                                         start=False, stop=True)
                    # evacuate (+bias) in 512-col pieces; emitted before
                    # next sparsemax so DVE frees og psum slots promptly
                    for h in halves:
                        o0 = h * 1024
                        osb = outpool.tile([128, 1024], f32, tag="osb",
                                           name="osb")
                        for s in (0, 512):
                            nc.vector.tensor_tensor(
                                osb[:, s:s + 512], accs[h][:, s:s + 512],
                                bb[:, o0 + s:o0 + s + 512], op=Add)
                            nc.sync.dma_start(
                                out_d[i * 128:(i + 1) * 128,
                                      o0 + s:o0 + s + 512],
                                osb[:, s:s + 512])
                del xts[i]

    nc.compile()
    _CACHE["nc"] = nc
    return nc


def make_in_maps(x, W_base, b_base, W_router, b_router, lora_A, lora_B):
    """Host-side packing (untimed): transposed/bf16 layouts per core."""
    bft = ml_dtypes.bfloat16
    xf = np.asarray(x, dtype=np.float32).reshape(B * S, D)
    # per-quarter x^T tiles: xt[i, p, c, t] = x_q[i*128+t, c*128+p]
    xts = []
    for q in range(TQ):
        xq = xf[q * T_CORE:(q + 1) * T_CORE]
        xt = xq.reshape(NT, 128, NC_D, 128).transpose(0, 3, 2, 1)
        xts.append(np.ascontiguousarray(xt, dtype=bft).reshape(
            NT, 128, NC_D * 128))
    # W^T halves: wt[p, c, o] = W_h[o, c*128+p]
    wts_h = []
    bbs = []
    bcats = []
    lbf = np.asarray(lora_B, dtype=np.float32).reshape(ER, O)
    for h in range(OH):
        Wh = np.asarray(W_base[h * O_CORE:(h + 1) * O_CORE], dtype=np.float32)
        wt = Wh.reshape(O_CORE, NC_D, 128).transpose(2, 1, 0)
        wts_h.append(np.ascontiguousarray(wt, dtype=bft))
        bh = np.asarray(b_base[h * O_CORE:(h + 1) * O_CORE], dtype=np.float32)
        bbs.append(np.ascontiguousarray(
            np.broadcast_to(bh, (128, O_CORE)), dtype=np.float32))
        bcats.append(np.ascontiguousarray(
            lbf[:, h * O_CORE:(h + 1) * O_CORE], dtype=bft))
    # router + lora_A combined rhs: comb[p, c, 0:8]=Wr^T, [p,c,8:136]=A^T
    wr = np.asarray(W_router, dtype=np.float32)
    wr_p = wr.reshape(E, NC_D, 128).transpose(2, 1, 0)       # [128, c, E]
    la = np.asarray(lora_A, dtype=np.float32)
    la_p = la.reshape(E, NC_D, 128, R).transpose(2, 1, 0, 3)  # [128, c, E, R]
    comb = np.concatenate(
        [wr_p, la_p.reshape(128, NC_D, ER)], axis=2)
    comb = np.ascontiguousarray(comb, dtype=bft)
    brb = np.ascontiguousarray(
        np.broadcast_to(np.asarray(b_router, dtype=np.float32), (128, E)))
    kbh = np.ascontiguousarray(np.broadcast_to(
        np.arange(1, E + 1, dtype=np.float32), (128, E)))
    ident = np.eye(128, dtype=bft)

    in_maps = []
    for core in range(N_CORES):
        q, h = core % TQ, core // TQ
        in_maps.append({
            "xt": xts[q],
            "wt": wts_h[h],
            "comb": comb,
            "bcat": bcats[h],
            "bb": bbs[h],
            "brb": brb,
            "kb": kbh,
            "ident": ident,
        })
    return in_maps


def assemble(results):
    out = np.empty((B * S, O), dtype=np.float32)
    for core in range(N_CORES):
        q, h = core % TQ, core // TQ
        out[q * T_CORE:(q + 1) * T_CORE,
            h * O_CORE:(h + 1) * O_CORE] = results[core]["out"]
    return out.reshape(B, S, O)


def kernel(x, W_base, b_base, W_router, b_router, lora_A, lora_B):
    nc = _build()
    in_maps = make_in_maps(x, W_base, b_base, W_router, b_router,
                           lora_A, lora_B)
    res = run_bass_kernel_spmd(nc, in_maps, core_ids=list(range(N_CORES)))
    return assemble(res.results)


if __name__ == "__main__":
    _build()
    print("kernel build+compile OK")


# revision 42
# speedup vs baseline: 1.0494x; 1.0494x over previous
"""Trainium2 Bass kernel for DGMoLE (dense-gated mixture of LoRA experts).

Computes, for x:[B,S,Din], W_base:[Dout,Din], b_base:[Dout], W_router:[E,Din],
b_router:[E], lora_A:[E,Din,R], lora_B:[E,R,Dout]:

    base   = x @ W_base.T + b_base
    wts    = sparsemax(x @ W_router.T + b_router)
    h      = einsum('td,edr->ter', x, lora_A)
    out    = base + einsum('ter,te,ero->to', h, wts, lora_B)

Sharding over 8 NeuronCores: 4 token-quarters x 2 Dout-halves.

All operand layout work is done host-side in numpy (untimed):
x is pre-transposed into per-tile [d,t] bf16 blocks, W into [d,o] bf16,
router+lora_A packed into one [d, 8+128] rhs, biases pre-broadcast.
The device program is pure matmul streaming: per 128-token tile, one
c-loop of 32x(4 matmuls of 512 cols) accumulating base output.  The
router/h chains run as a cursor-driven pipeline up to 2-3 tiles ahead
(each finished chain is copied from its single PSUM bank to SBUF, and
its sparsemax runs on the DVE under the next og loop), so the lora_B
matmuls that close each tile's accumulation never stall the PE.  No PE
transposes of x/W, no dtype-converting DMA, no startup W build; dummy
matmuls on a memset tile keep the HAM clock warm through the initial
DMA window, and the last tile runs as two o-half passes to shorten the
program tail.
"""

import sys

sys.path.insert(0, "/opt/trn_rl_repo")

import numpy as np
import ml_dtypes

from concourse import bacc, tile, mybir
from concourse.bass_utils import run_bass_kernel_spmd

f32 = mybir.dt.float32
bf16 = mybir.dt.bfloat16
Add = mybir.AluOpType.add
Mult = mybir.AluOpType.mult
Max = mybir.AluOpType.max
Min = mybir.AluOpType.min
IsGt = mybir.AluOpType.is_gt
Sub = mybir.AluOpType.subtract

# Problem dims (hardcoded per spec).
B, S, D, O = 8, 2048, 4096, 4096
E, R = 8, 16
ER = E * R  # 128
N_CORES = 8
TQ = 4          # token quarters
OH = 2          # output halves
T_CORE = B * S // TQ      # 4096 tokens per core
O_CORE = O // OH          # 2048 output dims per core
NT = T_CORE // 128        # 32 token tiles
NC_D = D // 128           # 32 contraction chunks
RH = E + ER               # router+h fused rhs width = 136

_CACHE = {}


def _build(trace_sim=False):
    if "nc" in _CACHE:
        return _CACHE["nc"]

    nc = bacc.Bacc("TRN2", target_bir_lowering=False, debug=False,
                   num_devices=N_CORES)
    # Host-packed inputs (see make_in_maps).
    xt_d = nc.dram_tensor("xt", [NT, 128, NC_D * 128], bf16,
                          kind="ExternalInput").ap()   # [tile, d_in_chunk, (c,t)]
    wt_d = nc.dram_tensor("wt", [128, NC_D, O_CORE], bf16,
                          kind="ExternalInput").ap()   # [d_in_chunk, c, o]
    comb_d = nc.dram_tensor("comb", [128, NC_D, RH], bf16,
                            kind="ExternalInput").ap()  # [d, c, (router|loraA)]
    bcat_d = nc.dram_tensor("bcat", [ER, O_CORE], bf16,
                            kind="ExternalInput").ap()
    bb_d = nc.dram_tensor("bb", [128, O_CORE], f32, kind="ExternalInput").ap()
    brb_d = nc.dram_tensor("brb", [128, E], f32, kind="ExternalInput").ap()
    kb_d = nc.dram_tensor("kb", [128, E], f32, kind="ExternalInput").ap()
    id_d = nc.dram_tensor("ident", [128, 128], bf16, kind="ExternalInput").ap()
    out_d = nc.dram_tensor("out", [T_CORE, O_CORE], f32,
                           kind="ExternalOutput").ap()

    with tile.TileContext(nc, trace_sim=trace_sim) as tc:
        with (
            tc.tile_pool(name="const", bufs=1) as cpool,
            tc.tile_pool(name="xt", bufs=5) as xtpool,
            tc.tile_pool(name="small", bufs=2) as smpool,
            tc.tile_pool(name="outs", bufs=3) as outpool,
            tc.tile_pool(name="ps", bufs=1, space="PSUM") as pspool,
        ):
            # ---------------- constants (all straight DMA) ----------------
            # Issue order matters at startup: rh(0) needs xt0+comb, og(0)
            # needs wt chunks in order; everything shares ~350 GB/s.
            def load_xt(i, split=1):
                t = xtpool.tile([128, NC_D * 128], bf16, tag="xt", name="xtt")
                step = NC_D * 128 // split
                for s in range(split):
                    nc.sync.dma_start(t[:, s * step:(s + 1) * step],
                                      xt_d[i, :, s * step:(s + 1) * step])
                return t

            ident = cpool.tile([128, 128], bf16)
            nc.sync.dma_start(ident[:], id_d[:])
            xts = {}
            comb = cpool.tile([128, NC_D * RH], bf16)
            xt0 = xtpool.tile([128, NC_D * 128], bf16, tag="xt", name="xtt")
            for g in range(2):
                nc.sync.dma_start(xt0[:, g * 2048:(g + 1) * 2048],
                                  xt_d[0, :, g * 2048:(g + 1) * 2048])
                nc.sync.dma_start(
                    comb[:, g * 16 * RH:(g + 1) * 16 * RH].rearrange(
                        "p (c f) -> p c f", f=RH),
                    comb_d[:, g * 16:(g + 1) * 16, :])
            xts[0] = xt0
            xts[1] = load_xt(1)
            xts[2] = load_xt(2)

            # W^T chunks: one tile per contraction chunk so matmuls only
            # wait on the chunk they read (og(0) starts once chunk 0 lands).
            wt = []
            for c in range(NC_D):
                wt_c = cpool.tile([128, O_CORE], bf16, tag=f"wt{c}",
                                  name=f"wt{c}")
                nc.gpsimd.dma_start(wt_c[:], wt_d[:, c, :])
                wt.append(wt_c)
            bcat = cpool.tile([128, O_CORE], bf16)
            nc.sync.dma_start(bcat[:], bcat_d[:])
            bb = cpool.tile([128, O_CORE], f32)
            nc.sync.dma_start(bb[:], bb_d[:])
            xts[3] = load_xt(3)
            brb = cpool.tile([128, E], f32)
            nc.sync.dma_start(brb[:], brb_d[:])
            kb = cpool.tile([128, E], f32)
            nc.sync.dma_start(kb[:], kb_d[:])
            rh_ps = {}
            hw_tiles = {}

            def emit_sparsemax(i, rh):
                """DVE chain: rh (SBUF copy) logits -> weights -> hw."""
                z = smpool.tile([128, E], f32, tag="z", name="z")
                nc.vector.tensor_tensor(z[:], rh[:, 0:E], brb[:], op=Add)
                zs = smpool.tile([128, E], f32, tag="zs", name="zs")
                nc.vector.tensor_copy(zs[:], z[:])
                tmp = smpool.tile([128, E], f32, tag="tmp", name="tmp")
                # Batcher sort-8 (descending), comparators batched per stage
                # via strided access patterns: 6 stages x 3 ops.
                z2 = zs[:].rearrange("p (f two) -> p f two", two=2)
                z4 = zs[:].rearrange("p (g two f) -> p g two f", two=2, f=2)
                z8 = zs[:].rearrange("p (g f) -> p g f", f=4)
                z6 = zs[:, 1:7].rearrange("p (f two) -> p f two", two=2)
                t41 = tmp[:, 0:4].rearrange("p (f one) -> p f one", one=1)
                t42 = tmp[:, 0:4].rearrange("p (g one f) -> p g one f",
                                            one=1, f=2)
                t21 = tmp[:, 0:2].rearrange("p (f one) -> p f one", one=1)
                t31 = tmp[:, 0:3].rearrange("p (f one) -> p f one", one=1)
                stages = [
                    (z2[:, :, 0:1], z2[:, :, 1:2], t41),        # (01)(23)(45)(67)
                    (z4[:, :, 0:1, :], z4[:, :, 1:2, :], t42),  # (02)(13)(46)(57)
                    (z8[:, :, 1:2], z8[:, :, 2:3], t21),        # (12)(56)
                    (zs[:, 0:4], zs[:, 4:8], tmp[:, 0:4]),      # (04)(15)(26)(37)
                    (zs[:, 2:4], zs[:, 4:6], tmp[:, 0:2]),      # (24)(35)
                    (z6[:, :, 0:1], z6[:, :, 1:2], t31),        # (12)(34)(56)
                ]
                for (a_, b_, t_) in stages:
                    nc.vector.tensor_tensor(t_, a_, b_, op=Min)
                    nc.vector.tensor_tensor(a_, a_, b_, op=Max)
                    nc.vector.tensor_copy(b_, t_)
                # inclusive prefix sum along E (doubling, 6 ops)
                c1 = smpool.tile([128, E], f32, tag="c1", name="c1")
                c2 = smpool.tile([128, E], f32, tag="c2", name="c2")
                cum = smpool.tile([128, E], f32, tag="cum", name="cum")
                nc.vector.tensor_tensor(c1[:, 1:8], zs[:, 1:8], zs[:, 0:7],
                                        op=Add)
                nc.vector.tensor_copy(c1[:, 0:1], zs[:, 0:1])
                nc.vector.tensor_tensor(c2[:, 2:8], c1[:, 2:8], c1[:, 0:6],
                                        op=Add)
                nc.vector.tensor_copy(c2[:, 0:2], c1[:, 0:2])
                nc.vector.tensor_tensor(cum[:, 4:8], c2[:, 4:8], c2[:, 0:4],
                                        op=Add)
                nc.vector.tensor_copy(cum[:, 0:4], c2[:, 0:4])
                # support: 1 + k*z_(k) > cum_k
                kz1 = smpool.tile([128, E], f32, tag="kz1", name="kz1")
                nc.vector.tensor_tensor(kz1[:], zs[:], kb[:], op=Mult)
                nc.vector.tensor_scalar_add(kz1[:], kz1[:], 1.0)
                supp = smpool.tile([128, E], f32, tag="supp", name="supp")
                nc.vector.tensor_tensor(supp[:], kz1[:], cum[:], op=IsGt)
                kz = smpool.tile([128, 1], f32, tag="kz", name="kz")
                nc.vector.tensor_reduce(kz[:], supp[:],
                                        axis=mybir.AxisListType.X, op=Add)
                zsup = smpool.tile([128, E], f32, tag="zsup", name="zsup")
                tsum = smpool.tile([128, 1], f32, tag="tsum", name="tsum")
                nc.vector.tensor_tensor(zsup[:], zs[:], supp[:], op=Mult)
                nc.vector.tensor_reduce(tsum[:], zsup[:],
                                        axis=mybir.AxisListType.X, op=Add)
                nc.vector.tensor_scalar_add(tsum[:], tsum[:], -1.0)
                rk = smpool.tile([128, 1], f32, tag="rk", name="rk")
                nc.vector.reciprocal(rk[:], kz[:])
                tau = smpool.tile([128, 1], f32, tag="tau", name="tau")
                nc.vector.tensor_tensor(tau[:], tsum[:], rk[:], op=Mult)
                wts = smpool.tile([128, E], f32, tag="wts", name="wts")
                nc.vector.tensor_scalar(wts[:], z[:], tau[:], 0.0,
                                        op0=Sub, op1=Max)
                # hw = h * wts (bf16): one op, wts broadcast across ranks
                hw = smpool.tile([128, ER], bf16, tag="hw", name="hw",
                                 bufs=4)
                nc.vector.tensor_tensor(
                    hw[:].rearrange("p (e r) -> p e r", r=R),
                    rh[:, E:RH].rearrange("p (e r) -> p e r", r=R),
                    wts[:].rearrange("p (e one) -> p e one",
                                     one=1).broadcast_to([128, E, R]),
                    op=Mult)
                hw_tiles[i] = hw

            # ---------------- rh chain machinery ----------------
            # Router/h chains run as a cursor-driven pipeline up to 2 tiles
            # ahead of the og loop.  Each completed chain is copied to SBUF
            # (freeing the single rh psum bank) and its sparsemax emitted.
            cursor = [0, 0]

            def emit_rh_mm():
                j, k = cursor
                if k == 0:
                    rh_ps[j] = pspool.tile([128, RH], f32, tag="rh", bufs=1,
                                           name="rhn")
                nc.tensor.matmul(rh_ps[j][:], xts[j][:, k * 128:(k + 1) * 128],
                                 comb[:, k * RH:(k + 1) * RH],
                                 start=(k == 0), stop=(k == NC_D - 1))
                if k == NC_D - 1:
                    rhc = smpool.tile([128, RH], f32, tag="rhc", name="rhc")
                    nc.vector.tensor_copy(rhc[:], rh_ps[j][:])
                    emit_sparsemax(j, rhc)
                    cursor[0], cursor[1] = j + 1, 0
                else:
                    cursor[1] = k + 1

            # ---------------- prologue ----------------
            # PE warm-up: dummy transposes on a memset tile (no DMA dep)
            # keep the HAM activity window busy from ~0.5us until the first
            # input DMAs land, so rh(0)/og(0) run at 2.4 GHz, not 1.2.
            junk = cpool.tile([128, 128], bf16, name="junk")
            nc.vector.memset(junk[:], 0.0)
            warm = pspool.tile([128, 128], f32, tag="tr", bufs=1,
                               name="warm")
            for _ in range(120):
                nc.tensor.matmul(warm[:], junk[:], junk[:],
                                 start=True, stop=True)

            # ---------------- main token loop ----------------
            for i in range(NT):
                if i + 4 < NT:
                    xts[i + 4] = load_xt(i + 4)
                xt_i = xts[i]
                hwT = None
                rh_cap = min(i + 3 if i == 0 else i + 2, NT - 1)
                hw_c = 22 if i == 0 else 14
                # The last tile runs as two sequential o-half passes so the
                # first half's evacuation+DMA overlaps the second's matmuls,
                # trimming the program tail.
                passes = [(0,), (1,)] if i == NT - 1 else [(0, 1)]
                for halves in passes:
                    accs = {
                        h: pspool.tile([128, 1024], f32, tag="og", bufs=3,
                                       name=f"acc{h}")
                        for h in halves
                    }
                    for c in range(NC_D):
                        lhs = xt_i[:, c * 128:(c + 1) * 128]
                        w_c = wt[c]
                        st = (c == 0)
                        for h in halves:
                            o0 = h * 1024
                            nc.tensor.matmul(
                                accs[h][:, 0:512], lhs,
                                w_c[:, o0:o0 + 512], start=st, stop=False)
                            nc.tensor.matmul(
                                accs[h][:, 512:1024], lhs,
                                w_c[:, o0 + 512:o0 + 1024],
                                start=st, stop=False)
                        if c == hw_c and hwT is None:
                            # transpose hw(i) for the lora_B matmuls below
                            trp = pspool.tile([128, 128], bf16, tag="tr",
                                              bufs=1, name="trp")
                            nc.tensor.transpose(trp[:], hw_tiles[i][:],
                                                ident[:])
                            hwT = smpool.tile([128, ER], bf16, tag="hwT",
                                              name="hwT")
                            nc.vector.tensor_copy(hwT[:], trp[:])
                        if c >= (1 if i == 0 else 4):
                            behind = cursor[0] <= i + 1
                            quota = (5 if i == 0 else 3) if behind else 2
                            n = 0
                            while cursor[0] <= rh_cap and n < quota:
                                emit_rh_mm()
                                n += 1
                    # close base accumulation with the lora_B contribution
                    for h in halves:
                        o0 = h * 1024
                        nc.tensor.matmul(accs[h][:, 0:512], hwT[:],
                                         bcat[:, o0:o0 + 512],
                                         start=False, stop=True)
                        nc.tensor.matmul(accs[h][:, 512:1024], hwT[:],
                                         bcat[:, o0 + 512:o0 + 1024],
                                         start=False, stop=True)
                    # evacuate (+bias) in 512-col pieces; emitted before
                    # next sparsemax so DVE frees og psum slots promptly
                    for h in halves:
                        o0 = h * 1024
                        osb = outpool.tile([128, 1024], f32, tag="osb",
                                           name="osb")
                        for s in (0, 512):
                            nc.vector.tensor_tensor(
                                osb[:, s:s + 512], accs[h][:, s:s + 512],
                                bb[:, o0 + s:o0 + s + 512], op=Add)
                            nc.sync.dma_start(
                                out_d[i * 128:(i + 1) * 128,
                                      o0 + s:o0 + s + 512],
                                osb[:, s:s + 512])
                del xts[i]

    nc.compile()
    _CACHE["nc"] = nc
    return nc


def make_in_maps(x, W_base, b_base, W_router, b_router, lora_A, lora_B):
    """Host-side packing (untimed): transposed/bf16 layouts per core."""
    bft = ml_dtypes.bfloat16
    xf = np.asarray(x, dtype=np.float32).reshape(B * S, D)
    # per-quarter x^T tiles: xt[i, p, c, t] = x_q[i*128+t, c*128+p]
    xts = []
    for q in range(TQ):
        xq = xf[q * T_CORE:(q + 1) * T_CORE]
        xt = xq.reshape(NT, 128, NC_D, 128).transpose(0, 3, 2, 1)
        xts.append(np.ascontiguousarray(xt, dtype=bft).reshape(
            NT, 128, NC_D * 128))
    # W^T halves: wt[p, c, o] = W_h[o, c*128+p]
    wts_h = []
    bbs = []
    bcats = []
    lbf = np.asarray(lora_B, dtype=np.float32).reshape(ER, O)
    for h in range(OH):
        Wh = np.asarray(W_base[h * O_CORE:(h + 1) * O_CORE], dtype=np.float32)
        wt = Wh.reshape(O_CORE, NC_D, 128).transpose(2, 1, 0)
        wts_h.append(np.ascontiguousarray(wt, dtype=bft))
        bh = np.asarray(b_base[h * O_CORE:(h + 1) * O_CORE], dtype=np.float32)
        bbs.append(np.ascontiguousarray(
            np.broadcast_to(bh, (128, O_CORE)), dtype=np.float32))
        bcats.append(np.ascontiguousarray(
            lbf[:, h * O_CORE:(h + 1) * O_CORE], dtype=bft))
    # router + lora_A combined rhs: comb[p, c, 0:8]=Wr^T, [p,c,8:136]=A^T
    wr = np.asarray(W_router, dtype=np.float32)
    wr_p = wr.reshape(E, NC_D, 128).transpose(2, 1, 0)       # [128, c, E]
    la = np.asarray(lora_A, dtype=np.float32)
    la_p = la.reshape(E, NC_D, 128, R).transpose(2, 1, 0, 3)  # [128, c, E, R]
    comb = np.concatenate(
        [wr_p, la_p.reshape(128, NC_D, ER)], axis=2)
    comb = np.ascontiguousarray(comb, dtype=bft)
    brb = np.ascontiguousarray(
        np.broadcast_to(np.asarray(b_router, dtype=np.float32), (128, E)))
    kbh = np.ascontiguousarray(np.broadcast_to(
        np.arange(1, E + 1, dtype=np.float32), (128, E)))
    ident = np.eye(128, dtype=bft)

    in_maps = []
    for core in range(N_CORES):
        q, h = core % TQ, core // TQ
        in_maps.append({
            "xt": xts[q],
            "wt": wts_h[h],
            "comb": comb,
            "bcat": bcats[h],
            "bb": bbs[h],
            "brb": brb,
            "kb": kbh,
            "ident": ident,
        })
    return in_maps


def assemble(results):
    out = np.empty((B * S, O), dtype=np.float32)
    for core in range(N_CORES):
        q, h = core % TQ, core // TQ
        out[q * T_CORE:(q + 1) * T_CORE,
            h * O_CORE:(h + 1) * O_CORE] = results[core]["out"]
    return out.reshape(B, S, O)


def kernel(x, W_base, b_base, W_router, b_router, lora_A, lora_B):
    nc = _build()
    in_maps = make_in_maps(x, W_base, b_base, W_router, b_router,
                           lora_A, lora_B)
    res = run_bass_kernel_spmd(nc, in_maps, core_ids=list(range(N_CORES)))
    return assemble(res.results)


if __name__ == "__main__":
    _build()
    print("kernel build+compile OK")
